# revision 23
# baseline (speedup 1.0000x reference)
"""NetVLAD forward on 8 Trainium2 NeuronCores.

Reference computation (per batch b):
    logits = conv_w @ x_flat[b]            # [K, N]    (K=64, C=128, N=4096)
    a      = softmax(logits, axis=K)
    vlad   = a @ x_flat[b].T - sum_n(a) * centroids    # [K, C]
    vlad   = l2norm(vlad, axis=C); out[b] = l2norm(vlad.reshape(K*C))

Sharding: data-parallel over batch (8 per core), weights replicated.

Device computes per batch vladT = sum_n (x*r)[:,n] e[n,:] in [C, K] and
asum[k] = sum_n a[n,k]; the tiny epilogue (centroid subtraction + two L2
norms, <1% FLOPs) runs on the host after the gather.

Structure (driven by the TimelineSim cost model, where a matmul costs
out-free-size rows and stationary loads are free):
  - mm1  (PE): logits[n,K] = x_chunk^T @ w             -> 64 rows/chunk
  - ACT: e = exp(logits) psum->sbuf (bf16, k-minor)
  - DVE: s = sum_k e (fp32, per half-batch), r = 1/s, r2 = bf16 pairs
  - xT materialization, split per 8-chunk bank to balance engines:
      banks 0,1: PE transpose -> psum, DVE fused copy+scale (2x perf mode
                 via an innermost packed [1,2] pair AP over duplicated r2)
      bank 2:    PE transpose -> psum, ACT plain copy; Pool scales e->a
      chunks 24..31: one DMA-transpose (xbar, 14ns/tile) straight from the
                 x SBUF tile into xts; Pool scales e->a. Chunk j covers the
                 strided column set {3072+8p+j} - consistent everywhere.
  - mm2' (PE): vladT[C,K] += xts_chunk^T @ (e|a)_chunk -> 64 rows/chunk
  - asum (PE): pa[K,1] += (e^T r2 | a^T ones)          -> 1 row/chunk
Two-batch software pipelining (phase2 lags phase1 by 2) keeps the PE dense
(p-state ramp to 2.4 GHz); output stores issue from the ACT engine's DGE so
they never head-of-line block the SP load queue.

Softmax skips max-subtraction: logits ~ N(0, 1.3), |logit| < 8 for this
input distribution, exp() stays comfortably in range.
"""

import numpy as np
import ml_dtypes
from contextlib import ExitStack

import concourse.bass as bass
import concourse.bacc as bacc
import concourse.tile as tile
import concourse.mybir as mybir
from concourse import bass_utils

B, C, K = 64, 128, 64
HW = 64 * 64  # N = H*W
NCORES = 8
BPC = B // NCORES  # batches per core
F32 = mybir.dt.float32
BF16 = mybir.dt.bfloat16

NCHUNK = 128            # n-columns per chunk (PE contraction limit)
NCH = HW // NCHUNK      # chunks per batch = 32
G = 8                   # chunks per psum bank / exp group
NG = NCH // G           # groups per batch = 4

# per-bank xT escape route (see module docstring)
BANKS = ("dve", "dve", "act", "dmat")
N_DMAT = sum(G for b in BANKS if b == "dmat")
NPT = NCH - N_DMAT      # PE-transposed chunks
DMAT_LO = NPT * NCHUNK  # first x column handled by the DMA transpose


def _pair_view(t_ap):
    """[128, G, C] AP -> [128, G, C/2, 2] (same memory, innermost packed pair)."""
    p, g, c = t_ap.ap
    return bass.AP(
        tensor=t_ap.tensor,
        offset=t_ap.offset,
        ap=[p, g, [2 * c[0], c[1] // 2], [c[0], 2]],
    )


def _r2_bcast(r2_ap, n_c):
    """r2 [128, M, 2] AP -> [128, M, n_c/2, 2]: broadcast the duplicated pair
    over c via a stride-0 dim, keeping the innermost dim packed so the DVE
    2x perf mode survives."""
    p, m, two = r2_ap.ap
    return bass.AP(
        tensor=r2_ap.tensor,
        offset=r2_ap.offset,
        ap=[p, m, [0, n_c // 2], two],
    )


def _bcast_k(r2_ap, n_k):
    """r2 [128, M, 2] AP -> [128, M, n_k] broadcasting slot 0 over k."""
    p, m, _two = r2_ap.ap
    return bass.AP(tensor=r2_ap.tensor, offset=r2_ap.offset, ap=[p, m, [0, n_k]])


def _dup2(r_ap):
    """r [128, M] AP -> [128, M, 2] input view repeating each value twice."""
    p, m = r_ap.ap
    return bass.AP(tensor=r_ap.tensor, offset=r_ap.offset, ap=[p, m, [0, 2]])


def _strided_cols(x_ap, base, stride, n):
    """x [C, HW] AP -> [C, n] AP of columns base, base+stride, ..."""
    p, cdim = x_ap.ap
    assert cdim[0] == 1
    return bass.AP(
        tensor=x_ap.tensor, offset=x_ap.offset + base, ap=[p, [stride, n]]
    )


def _netvlad_tile(tc: tile.TileContext, out_d, x_d, w_d, ident_d):
    nc = tc.nc
    out_v_d, out_a_d = out_d
    with ExitStack() as ctx:
        const = ctx.enter_context(tc.tile_pool(name="const", bufs=1))
        xpool = ctx.enter_context(tc.tile_pool(name="x", bufs=3))
        epool = ctx.enter_context(tc.tile_pool(name="e", bufs=3))
        rpool = ctx.enter_context(tc.tile_pool(name="r", bufs=3))
        spool = ctx.enter_context(tc.tile_pool(name="s", bufs=2))
        xtspool = ctx.enter_context(tc.tile_pool(name="xts", bufs=3))
        opool = ctx.enter_context(tc.tile_pool(name="o", bufs=2))
        pl_pool = ctx.enter_context(tc.tile_pool(name="pl", bufs=2, space="PSUM"))
        pt_pool = ctx.enter_context(tc.tile_pool(name="pt", bufs=3, space="PSUM"))
        pv_pool = ctx.enter_context(tc.tile_pool(name="pv", bufs=2, space="PSUM"))
        pa_pool = ctx.enter_context(tc.tile_pool(name="pa", bufs=1, space="PSUM"))

        w_sb = const.tile([C, K], BF16)
        nc.sync.dma_start(out=w_sb, in_=w_d)
        ident_sb = const.tile([C, C], BF16)
        nc.sync.dma_start(out=ident_sb, in_=ident_d)
        ones_sb = const.tile([NCHUNK, 1], BF16)
        nc.gpsimd.memset(ones_sb, 1.0)

        xs_tiles = [None] * BPC

        def load_x(ib):
            xt = xpool.tile([C, HW], BF16, tag="x")
            nc.sync.dma_start(out=xt, in_=x_d[ib])
            xs_tiles[ib] = xt

        def chunk_x_ap(x, ch):
            """lhsT AP for mm1 of chunk ch (matches the xts n-mapping)."""
            if ch * NCHUNK < DMAT_LO:
                return x[:, ch * NCHUNK : (ch + 1) * NCHUNK]
            j = ch - NPT
            return _strided_cols(x, DMAT_LO + j, N_DMAT, NCHUNK)

        saved = [None] * BPC

        def phase1(ib):
            x = xs_tiles[ib]
            e = epool.tile([NCHUNK, NCH, K], BF16, tag="e")
            a = epool.tile([NCHUNK, NCH - 2 * G, K], BF16, tag="a")
            xts = xtspool.tile([NCHUNK, NCH, C], BF16, tag="xts")
            r2 = rpool.tile([NCHUNK, NCH, 2], BF16, tag="r")

            # DMA-transposed tail: xbar 16x128 tiles from the x sbuf tile
            nc.sync.dma_start_transpose(
                out=xts[:, NPT:NCH, :], in_=x[:, DMAT_LO:HW]
            )

            pts = []
            for g in range(NG):
                pl = pl_pool.tile([NCHUNK, G, K], F32)
                pt = None
                if BANKS[g] != "dmat":
                    pt = pt_pool.tile([NCHUNK, G, C], BF16)
                for i in range(G):
                    ch = g * G + i
                    xsl = chunk_x_ap(x, ch)
                    nc.tensor.matmul(
                        pl[:, i, :], lhsT=xsl, rhs=w_sb, start=True, stop=True
                    )
                    if pt is not None:
                        nc.tensor.transpose(pt[:, i, :], in_=xsl, identity=ident_sb)
                nc.scalar.activation(
                    e[:, g * G : (g + 1) * G, :],
                    pl,
                    mybir.ActivationFunctionType.Exp,
                )
                pts.append(pt)

                if g % 2 == 1:  # softmax denominators per half-batch
                    h0 = (g - 1) * G
                    s = spool.tile([NCHUNK, 2 * G], F32, tag="s")
                    nc.vector.reduce_sum(
                        s, e[:, h0 : h0 + 2 * G, :], axis=mybir.AxisListType.X
                    )
                    r = spool.tile([NCHUNK, 2 * G], F32, tag="rf")
                    nc.vector.reciprocal(r, s)
                    nc.vector.tensor_copy(
                        out=r2[:, h0 : h0 + 2 * G, :], in_=_dup2(r)
                    )
                    # escape the pt banks of this half
                    for gg in (g - 1, g):
                        lo = gg * G
                        if BANKS[gg] == "dve":
                            nc.vector.tensor_tensor(
                                out=_pair_view(xts[:, lo : lo + G, :]),
                                in0=_pair_view(pts[gg]),
                                in1=_r2_bcast(r2[:, lo : lo + G, :], C),
                                op=mybir.AluOpType.mult,
                            )
                        elif BANKS[gg] == "act":
                            nc.scalar.copy(
                                out=xts[:, lo : lo + G, :], in_=pts[gg]
                            )

            # Pool scales e->a for all chunks whose xts is unscaled
            lo = 2 * G
            nc.gpsimd.tensor_tensor(
                out=a,
                in0=e[:, lo:NCH, :],
                in1=_bcast_k(r2[:, lo:NCH, :], K),
                op=mybir.AluOpType.mult,
            )
            saved[ib] = (e, a, r2, xts)

        def phase2(ib):
            e, a, r2, xts = saved[ib]
            pv = pv_pool.tile([C, K], F32)  # vladT
            pa = pa_pool.tile([K, 1], F32)  # asum
            for ch in range(NCH):
                g = ch // G
                if BANKS[g] == "dve":
                    mv_sl = e[:, ch, :]
                    asum_rhs = r2[:, ch, 0:1]
                else:
                    mv_sl = a[:, ch - 2 * G, :]
                    asum_rhs = ones_sb
                nc.tensor.matmul(
                    pv,
                    lhsT=xts[:, ch, :],
                    rhs=mv_sl,
                    start=(ch == 0),
                    stop=(ch == NCH - 1),
                )
                nc.tensor.matmul(
                    pa,
                    lhsT=mv_sl,
                    rhs=asum_rhs,
                    start=(ch == 0),
                    stop=(ch == NCH - 1),
                )
            outt = opool.tile([C, K], F32, tag="o")
            nc.scalar.copy(out=outt, in_=pv)
            nc.scalar.dma_start(out=out_v_d[ib], in_=outt)
            outa = opool.tile([K, 1], F32, tag="oa")
            nc.scalar.copy(out=outa, in_=pa)
            nc.scalar.dma_start(out=out_a_d[ib], in_=outa)
            saved[ib] = None

        with nc.allow_low_precision("bf16 softmax scales: ~0.4% rel err"):
            load_x(0)
            load_x(1)
            for ib in range(BPC):
                phase1(ib)
                if ib + 2 < BPC:
                    load_x(ib + 2)
                if ib >= 2:
                    phase2(ib - 2)
            phase2(BPC - 2)
            phase2(BPC - 1)


_NC_CACHE = None


def _get_nc():
    global _NC_CACHE
    if _NC_CACHE is None:
        nc = bacc.Bacc(
            "TRN2",
            target_bir_lowering=False,
            debug=False,
            num_devices=NCORES,
        )
        x_d = nc.dram_tensor("x", [BPC, C, HW], BF16, kind="ExternalInput").ap()
        w_d = nc.dram_tensor("w_t", [C, K], BF16, kind="ExternalInput").ap()
        ident_d = nc.dram_tensor("ident", [C, C], BF16, kind="ExternalInput").ap()
        out_v_d = nc.dram_tensor("out_v", [BPC, C, K], F32, kind="ExternalOutput").ap()
        out_a_d = nc.dram_tensor("out_a", [BPC, K, 1], F32, kind="ExternalOutput").ap()
        with tile.TileContext(nc) as tc:
            _netvlad_tile(tc, (out_v_d, out_a_d), x_d, w_d, ident_d)
        nc.compile()
        _NC_CACHE = nc
    return _NC_CACHE


def _make_in_maps(x, conv_w):
    bf16 = ml_dtypes.bfloat16
    x_flat = np.ascontiguousarray(x.reshape(B, C, HW).astype(bf16))
    w_t = np.ascontiguousarray(conv_w.T.astype(bf16))  # [C, K]
    ident = np.eye(C, dtype=np.float32).astype(bf16)
    in_maps = []
    for core in range(NCORES):
        in_maps.append(
            {
                "x": x_flat[core * BPC : (core + 1) * BPC],
                "w_t": w_t,
                "ident": ident,
            }
        )
    return in_maps


def _run(in_maps, trace=False, **kwargs):
    nc = _get_nc()
    return bass_utils.run_bass_kernel_spmd(
        nc, in_maps, core_ids=list(range(NCORES)), trace=trace, **kwargs
    )


def _postprocess(raw_v, raw_a, centroids):
    """raw_v: [B, C, K] vladT; raw_a: [B, K] asum -> [B, K*C] normalized."""
    vlad = raw_v.transpose(0, 2, 1) - raw_a[:, :, None] * centroids
    norms = np.sqrt((vlad * vlad).sum(axis=2, keepdims=True))
    vlad = vlad / np.maximum(norms, 1e-12)
    out = vlad.reshape(raw_v.shape[0], K * C)
    gn = np.sqrt((out * out).sum(axis=1, keepdims=True))
    return out / np.maximum(gn, 1e-12)


def kernel(x, conv_w, centroids):
    x = np.asarray(x)
    conv_w = np.asarray(conv_w)
    centroids = np.asarray(centroids, dtype=np.float32)
    res = _run(_make_in_maps(x, conv_w))
    raw_v = np.concatenate([r["out_v"] for r in res.results], axis=0)  # [B, C, K]
    raw_a = np.concatenate([r["out_a"] for r in res.results], axis=0)[:, :, 0]
    return _postprocess(
        raw_v.astype(np.float32), raw_a.astype(np.float32), centroids
    ).astype(np.float32)


# revision 26
# speedup vs baseline: 1.0176x; 1.0176x over previous
"""NetVLAD forward on 8 Trainium2 NeuronCores.

Reference computation (per batch b):
    logits = conv_w @ x_flat[b]            # [K, N]    (K=64, C=128, N=4096)
    a      = softmax(logits, axis=K)
    vlad   = a @ x_flat[b].T - sum_n(a) * centroids    # [K, C]
    vlad   = l2norm(vlad, axis=C); out[b] = l2norm(vlad.reshape(K*C))

Sharding: data-parallel over batch (8 per core), weights replicated.

Device computes per batch vladT = sum_n (x*r)[:,n] e[n,:] in [C, K] and
asum[k] = sum_n a[n,k]; the tiny epilogue (centroid subtraction + two L2
norms, <1% FLOPs) runs on the host after the gather.

Structure (driven by the TimelineSim cost model, where a matmul costs
out-free-size rows and stationary loads are free):
  - mm1  (PE): logits[n,K] = x_chunk^T @ w             -> 64 rows/chunk
  - ACT: e = exp(logits) psum->sbuf (bf16, k-minor)
  - DVE: s = sum_k e (fp32, per half-batch), r = 1/s, r2 = bf16 pairs
  - xT materialization, split per 8-chunk bank to balance engines:
      banks 0,1: PE transpose -> psum, DVE fused copy+scale (2x perf mode
                 via an innermost packed [1,2] pair AP over duplicated r2)
      bank 2:    PE transpose -> psum, ACT plain copy; Pool scales e->a
      chunks 24..31: one DMA-transpose (xbar, 14ns/tile) straight from the
                 x SBUF tile into xts; Pool scales e->a. Chunk j covers the
                 strided column set {3072+8p+j} - consistent everywhere.
  - mm2' (PE): vladT[C,K] += xts_chunk^T @ (e|a)_chunk -> 64 rows/chunk
  - asum (PE): pa[K,1] += (e^T r2 | a^T ones)          -> 1 row/chunk
Two-batch software pipelining (phase2 lags phase1 by 2) keeps the PE dense
(p-state ramp to 2.4 GHz); output stores issue from the ACT engine's DGE so
they never head-of-line block the SP load queue.

Softmax skips max-subtraction: logits ~ N(0, 1.3), |logit| < 8 for this
input distribution, exp() stays comfortably in range.
"""

import numpy as np
import ml_dtypes
from contextlib import ExitStack

import concourse.bass as bass
import concourse.bacc as bacc
import concourse.tile as tile
import concourse.mybir as mybir
from concourse import bass_utils

B, C, K = 64, 128, 64
HW = 64 * 64  # N = H*W
NCORES = 8
BPC = B // NCORES  # batches per core
F32 = mybir.dt.float32
BF16 = mybir.dt.bfloat16

NCHUNK = 128            # n-columns per chunk (PE contraction limit)
NCH = HW // NCHUNK      # chunks per batch = 32
G = 8                   # chunks per psum bank / exp group
NG = NCH // G           # groups per batch = 4

# per-bank xT escape route (see module docstring)
BANKS = ("dve", "dve", "act", "dmat")
N_DMAT = sum(G for b in BANKS if b == "dmat")
NPT = NCH - N_DMAT      # PE-transposed chunks
DMAT_LO = NPT * NCHUNK  # first x column handled by the DMA transpose


def _pair_view(t_ap):
    """[128, G, C] AP -> [128, G, C/2, 2] (same memory, innermost packed pair)."""
    p, g, c = t_ap.ap
    return bass.AP(
        tensor=t_ap.tensor,
        offset=t_ap.offset,
        ap=[p, g, [2 * c[0], c[1] // 2], [c[0], 2]],
    )


def _r2_bcast(r2_ap, n_c):
    """r2 [128, M, 2] AP -> [128, M, n_c/2, 2]: broadcast the duplicated pair
    over c via a stride-0 dim, keeping the innermost dim packed so the DVE
    2x perf mode survives."""
    p, m, two = r2_ap.ap
    return bass.AP(
        tensor=r2_ap.tensor,
        offset=r2_ap.offset,
        ap=[p, m, [0, n_c // 2], two],
    )


def _bcast_k(r2_ap, n_k):
    """r2 [128, M, 2] AP -> [128, M, n_k] broadcasting slot 0 over k."""
    p, m, _two = r2_ap.ap
    return bass.AP(tensor=r2_ap.tensor, offset=r2_ap.offset, ap=[p, m, [0, n_k]])


def _dup2(r_ap):
    """r [128, M] AP -> [128, M, 2] input view repeating each value twice."""
    p, m = r_ap.ap
    return bass.AP(tensor=r_ap.tensor, offset=r_ap.offset, ap=[p, m, [0, 2]])


def _strided_cols(x_ap, base, stride, n):
    """x [C, HW] AP -> [C, n] AP of columns base, base+stride, ..."""
    p, cdim = x_ap.ap
    assert cdim[0] == 1
    return bass.AP(
        tensor=x_ap.tensor, offset=x_ap.offset + base, ap=[p, [stride, n]]
    )


def _netvlad_tile(tc: tile.TileContext, out_d, x_d, w_d, ident_d):
    nc = tc.nc
    out_v_d, out_a_d = out_d
    with ExitStack() as ctx:
        const = ctx.enter_context(tc.tile_pool(name="const", bufs=1))
        xpool = ctx.enter_context(tc.tile_pool(name="x", bufs=3))
        epool = ctx.enter_context(tc.tile_pool(name="e", bufs=3))
        rpool = ctx.enter_context(tc.tile_pool(name="r", bufs=3))
        spool = ctx.enter_context(tc.tile_pool(name="s", bufs=2))
        xtspool = ctx.enter_context(tc.tile_pool(name="xts", bufs=3))
        opool = ctx.enter_context(tc.tile_pool(name="o", bufs=2))
        pl_pool = ctx.enter_context(tc.tile_pool(name="pl", bufs=2, space="PSUM"))
        pt_pool = ctx.enter_context(tc.tile_pool(name="pt", bufs=2, space="PSUM"))
        pv_pool = ctx.enter_context(tc.tile_pool(name="pv", bufs=2, space="PSUM"))
        pa_pool = ctx.enter_context(tc.tile_pool(name="pa", bufs=2, space="PSUM"))

        w_sb = const.tile([C, K], BF16)
        nc.sync.dma_start(out=w_sb, in_=w_d)
        ident_sb = const.tile([C, C], BF16)
        nc.sync.dma_start(out=ident_sb, in_=ident_d)
        ones_sb = const.tile([NCHUNK, 1], BF16)
        nc.gpsimd.memset(ones_sb, 1.0)

        xs_tiles = [None] * BPC

        def load_x(ib):
            xt = xpool.tile([C, HW], BF16, tag="x")
            nc.sync.dma_start(out=xt, in_=x_d[ib])
            xs_tiles[ib] = xt

        def chunk_x_ap(x, ch):
            """lhsT AP for mm1 of chunk ch (matches the xts n-mapping)."""
            if ch * NCHUNK < DMAT_LO:
                return x[:, ch * NCHUNK : (ch + 1) * NCHUNK]
            j = ch - NPT
            return _strided_cols(x, DMAT_LO + j, N_DMAT, NCHUNK)

        saved = [None] * BPC

        def phase1(ib):
            x = xs_tiles[ib]
            e = epool.tile([NCHUNK, NCH, K], BF16, tag="e")
            a = epool.tile([NCHUNK, NCH - 2 * G, K], BF16, tag="a")
            xts = xtspool.tile([NCHUNK, NCH, C], BF16, tag="xts")
            r2 = rpool.tile([NCHUNK, NCH, 2], BF16, tag="r")

            # DMA-transposed tail: xbar 16x128 tiles from the x sbuf tile
            nc.sync.dma_start_transpose(
                out=xts[:, NPT:NCH, :], in_=x[:, DMAT_LO:HW]
            )

            pts = []
            for g in range(NG):
                pl = pl_pool.tile([NCHUNK, G, K], F32)
                pt = None
                if BANKS[g] != "dmat":
                    pt = pt_pool.tile([NCHUNK, G, C], BF16)
                for i in range(G):
                    ch = g * G + i
                    xsl = chunk_x_ap(x, ch)
                    nc.tensor.matmul(
                        pl[:, i, :], lhsT=xsl, rhs=w_sb, start=True, stop=True
                    )
                    if pt is not None:
                        nc.tensor.transpose(pt[:, i, :], in_=xsl, identity=ident_sb)
                nc.scalar.activation(
                    e[:, g * G : (g + 1) * G, :],
                    pl,
                    mybir.ActivationFunctionType.Exp,
                )
                pts.append(pt)

                if g % 2 == 1:  # softmax denominators per half-batch
                    h0 = (g - 1) * G
                    s = spool.tile([NCHUNK, 2 * G], F32, tag="s")
                    nc.vector.reduce_sum(
                        s, e[:, h0 : h0 + 2 * G, :], axis=mybir.AxisListType.X
                    )
                    r = spool.tile([NCHUNK, 2 * G], F32, tag="rf")
                    nc.vector.reciprocal(r, s)
                    nc.vector.tensor_copy(
                        out=r2[:, h0 : h0 + 2 * G, :], in_=_dup2(r)
                    )
                    # escape the pt banks of this half
                    for gg in (g - 1, g):
                        lo = gg * G
                        if BANKS[gg] == "dve":
                            nc.vector.tensor_tensor(
                                out=_pair_view(xts[:, lo : lo + G, :]),
                                in0=_pair_view(pts[gg]),
                                in1=_r2_bcast(r2[:, lo : lo + G, :], C),
                                op=mybir.AluOpType.mult,
                            )
                        elif BANKS[gg] == "act":
                            nc.scalar.copy(
                                out=xts[:, lo : lo + G, :], in_=pts[gg]
                            )

            # Pool scales e->a (one instr per bank) for unscaled-xts chunks
            for g in range(2, NG):
                lo = g * G
                nc.gpsimd.tensor_tensor(
                    out=a[:, lo - 2 * G : lo - G, :],
                    in0=e[:, lo : lo + G, :],
                    in1=_bcast_k(r2[:, lo : lo + G, :], K),
                    op=mybir.AluOpType.mult,
                )
            saved[ib] = (e, a, r2, xts)

        def phase2(ib):
            e, a, r2, xts = saved[ib]
            pv = pv_pool.tile([C, K], F32)  # vladT
            pa = pa_pool.tile([K, 1], F32)  # asum
            for ch in range(NCH):
                g = ch // G
                if BANKS[g] == "dve":
                    mv_sl = e[:, ch, :]
                    asum_rhs = r2[:, ch, 0:1]
                else:
                    mv_sl = a[:, ch - 2 * G, :]
                    asum_rhs = ones_sb
                nc.tensor.matmul(
                    pv,
                    lhsT=xts[:, ch, :],
                    rhs=mv_sl,
                    start=(ch == 0),
                    stop=(ch == NCH - 1),
                )
                nc.tensor.matmul(
                    pa,
                    lhsT=mv_sl,
                    rhs=asum_rhs,
                    start=(ch == 0),
                    stop=(ch == NCH - 1),
                )
            outt = opool.tile([C, K], F32, tag="o")
            nc.scalar.copy(out=outt, in_=pv)
            nc.scalar.dma_start(out=out_v_d[ib], in_=outt)
            outa = opool.tile([K, 1], F32, tag="oa")
            nc.scalar.copy(out=outa, in_=pa)
            nc.scalar.dma_start(out=out_a_d[ib], in_=outa)
            saved[ib] = None

        with nc.allow_low_precision("bf16 softmax scales: ~0.4% rel err"):
            load_x(0)
            load_x(1)
            for ib in range(BPC):
                if ib >= 2:
                    phase2(ib - 2)
                phase1(ib)
                if ib + 2 < BPC:
                    load_x(ib + 2)
            phase2(BPC - 2)
            phase2(BPC - 1)


_NC_CACHE = None


def _get_nc():
    global _NC_CACHE
    if _NC_CACHE is None:
        nc = bacc.Bacc(
            "TRN2",
            target_bir_lowering=False,
            debug=False,
            num_devices=NCORES,
        )
        x_d = nc.dram_tensor("x", [BPC, C, HW], BF16, kind="ExternalInput").ap()
        w_d = nc.dram_tensor("w_t", [C, K], BF16, kind="ExternalInput").ap()
        ident_d = nc.dram_tensor("ident", [C, C], BF16, kind="ExternalInput").ap()
        out_v_d = nc.dram_tensor("out_v", [BPC, C, K], F32, kind="ExternalOutput").ap()
        out_a_d = nc.dram_tensor("out_a", [BPC, K, 1], F32, kind="ExternalOutput").ap()
        with tile.TileContext(nc) as tc:
            _netvlad_tile(tc, (out_v_d, out_a_d), x_d, w_d, ident_d)
        nc.compile()
        _NC_CACHE = nc
    return _NC_CACHE


def _make_in_maps(x, conv_w):
    bf16 = ml_dtypes.bfloat16
    x_flat = np.ascontiguousarray(x.reshape(B, C, HW).astype(bf16))
    w_t = np.ascontiguousarray(conv_w.T.astype(bf16))  # [C, K]
    ident = np.eye(C, dtype=np.float32).astype(bf16)
    in_maps = []
    for core in range(NCORES):
        in_maps.append(
            {
                "x": x_flat[core * BPC : (core + 1) * BPC],
                "w_t": w_t,
                "ident": ident,
            }
        )
    return in_maps


def _run(in_maps, trace=False, **kwargs):
    nc = _get_nc()
    return bass_utils.run_bass_kernel_spmd(
        nc, in_maps, core_ids=list(range(NCORES)), trace=trace, **kwargs
    )


def _postprocess(raw_v, raw_a, centroids):
    """raw_v: [B, C, K] vladT; raw_a: [B, K] asum -> [B, K*C] normalized."""
    vlad = raw_v.transpose(0, 2, 1) - raw_a[:, :, None] * centroids
    norms = np.sqrt((vlad * vlad).sum(axis=2, keepdims=True))
    vlad = vlad / np.maximum(norms, 1e-12)
    out = vlad.reshape(raw_v.shape[0], K * C)
    gn = np.sqrt((out * out).sum(axis=1, keepdims=True))
    return out / np.maximum(gn, 1e-12)


def kernel(x, conv_w, centroids):
    x = np.asarray(x)
    conv_w = np.asarray(conv_w)
    centroids = np.asarray(centroids, dtype=np.float32)
    res = _run(_make_in_maps(x, conv_w))
    raw_v = np.concatenate([r["out_v"] for r in res.results], axis=0)  # [B, C, K]
    raw_a = np.concatenate([r["out_a"] for r in res.results], axis=0)[:, :, 0]
    return _postprocess(
        raw_v.astype(np.float32), raw_a.astype(np.float32), centroids
    ).astype(np.float32)
